# revision 1
# baseline (speedup 1.0000x reference)
"""DCGRU cell on 8 Trainium2 NeuronCores (Bass/Tile SPMD kernel).

Strategy (node sharding):
  - Nodes padded 3000->3072, sharded 8x384 rows per core.
  - The two random-walk supports are never materialized:
      S1 @ y = A^T @ (dinv  * y)   (dinv  = 1/rowsum(A))
      S2 @ y = A   @ (d2inv * y)   (d2inv = 1/colsum(A))
    Each core keeps two SBUF-resident stationary slices, pre-scaled on device:
      acols[n, j]  = A[n, cRL+j] * dinv[n]    (lhsT for S1-type products)
      arowsT[n, j] = A[cRL+j, n] * d2inv[n]   (lhsT for S2-type products)
    Degree sums need a tiny AllReduce of per-core partial sums.
  - Diffusion (orientation A): out[m,cb] = sum_n lhsT[n,m] * rhs[n,cb] with
    rhs = full x tensor [3072, 1056] streamed k-tile by k-tile from DRAM;
    Chebyshev step 2 needs the full x1 -> AllGather between steps.
  - Activations layout: natural [node, (b,c)] with col = b*66+c.  The
    projection contracts over (c,k-mat) so per-(b, mat) 128x66 blocks are
    transposed on the PE into xsT_b [330(+pad), 384], then W~ (host-permuted
    W rows k*66+c) projects in 3 k-tile matmuls.
  - All matmuls in float32r (TF32-class, full PE rate; end-to-end error
    ~1e-4 absmax-relative, verified against fp32 reference).
"""
import sys
import time

for _p in ("/opt/trn_rl_repo",):
    if _p not in sys.path:
        sys.path.insert(0, _p)

import numpy as np


# ---------------------------------------------------------------- config

class Cfg:
    def __init__(self, N=3000, NP=3072, B=16, F=2, U=64, NCORES=8):
        self.N, self.NP, self.B, self.F, self.U, self.NCORES = N, NP, B, F, U, NCORES
        self.C = F + U                    # 66
        self.FD = self.B * self.C         # 1056
        self.NT = NP // 128               # k tiles over nodes
        self.RL = NP // NCORES            # local rows per core
        self.MT = self.RL // 128          # local m tiles
        assert NP % 128 == 0 and self.RL % 128 == 0
        self.NMATS = 5                    # x0, x1s1, x2s1, x1s2, x2s2
        self.KT3 = self.NMATS             # one 128-row k-block per mat (c-padded)
        # main free chunking of FD for diffusion matmuls: 512,512,tail
        self.CH = 512
        self.NCH = (self.FD // self.CH)           # 2 full chunks
        self.TAIL = self.FD - self.NCH * self.CH  # 32


CFG = Cfg()


# ---------------------------------------------------------------- device build

def build_nc(cfg: Cfg, reps: int = 1, no_cc: bool = False):
    import concourse.bass as bass
    import concourse.mybir as mybir
    import concourse.tile as tile
    from concourse import bacc

    r32 = mybir.dt.float32r
    f32 = mybir.dt.float32
    Alu = mybir.AluOpType
    Act = mybir.ActivationFunctionType

    NP, NT, RL, MT, B, C, U, F, FD = (cfg.NP, cfg.NT, cfg.RL, cfg.MT, cfg.B,
                                      cfg.C, cfg.U, cfg.F, cfg.FD)
    CH, NCH, TAIL = cfg.CH, cfg.NCH, cfg.TAIL
    KT3 = cfg.KT3
    NC8 = cfg.NCORES

    nc = bacc.Bacc("TRN2", target_bir_lowering=False, debug=False,
                   num_devices=NC8)

    # external inputs (per core)
    acols_d = nc.dram_tensor("acols", [NT, 128, RL], r32, kind="ExternalInput")
    arowsT_d = nc.dram_tensor("arowsT", [NT, 128, RL], r32, kind="ExternalInput")
    x0full_d = nc.dram_tensor("x0full", [NT, 128, FD], r32, kind="ExternalInput")
    x0loc_d = nc.dram_tensor("x0loc", [MT, 128, FD], f32, kind="ExternalInput")
    x0T_d = nc.dram_tensor("x0T", [C, B, RL], r32, kind="ExternalInput")
    hxT_d = nc.dram_tensor("hxT", [U, B, RL], f32, kind="ExternalInput")
    Wg_d = nc.dram_tensor("Wg", [KT3, 128, 2 * U], r32, kind="ExternalInput")
    Wc_d = nc.dram_tensor("Wc", [KT3, 128, U], r32, kind="ExternalInput")
    bg_d = nc.dram_tensor("bg", [2 * U, 1], f32, kind="ExternalInput")
    bc_d = nc.dram_tensor("bc", [U, 1], f32, kind="ExternalInput")
    ident_d = nc.dram_tensor("ident", [128, 128], r32, kind="ExternalInput")
    out_d = nc.dram_tensor("out", [U, B, RL], f32, kind="ExternalOutput")

    with tile.TileContext(nc) as tc:
        import contextlib
        ctx = contextlib.ExitStack()
        with ctx:
            const = ctx.enter_context(tc.tile_pool(name="const", bufs=1))
            matsp = ctx.enter_context(tc.tile_pool(name="mats", bufs=3))
            rhsp = ctx.enter_context(tc.tile_pool(name="rhsp", bufs=2))
            ttp = ctx.enter_context(tc.tile_pool(name="ttp", bufs=2))
            xstp = ctx.enter_context(tc.tile_pool(name="xstp", bufs=2))
            sgp = ctx.enter_context(tc.tile_pool(name="sgp", bufs=2))
            ftp = ctx.enter_context(tc.tile_pool(name="ftp", bufs=3))
            x0lp = ctx.enter_context(tc.tile_pool(name="x0lp", bufs=1))
            psp = ctx.enter_context(tc.tile_pool(name="psp", bufs=1, space="PSUM"))
            dram = ctx.enter_context(tc.tile_pool(name="dram", bufs=reps, space="DRAM"))

            # ---------------- resident SBUF tensors
            acols_sb = const.tile([128, NT, RL], r32)
            arowsT_sb = const.tile([128, NT, RL], r32)
            hxtu_sb = const.tile([128, B, RL], f32)     # rows 0:U hxT, U:128 u
            rhc_sb = const.tile([128, B, RL], f32)      # rows 0:U rh, U:128 c/out
            wg_sb = const.tile([128, KT3, 2 * U], r32)
            wc_sb = const.tile([128, KT3, U], r32)
            bg_sb = const.tile([2 * U, 1], f32)
            bc_sb = const.tile([U, 1], f32)
            ident = const.tile([128, 128], r32)
            dred = const.tile([128, 2 * NT], f32)
            dtmp = const.tile([128, 2 * NT], f32)
            dmask = const.tile([128, 2 * NT], f32)
            dinv_sb = const.tile([128, 2 * NT], f32)
            zero_sb = const.tile([128, RL], f32)

            # ---------------- DRAM bounce buffers
            red_in = dram.tile([128, 2 * NT], f32, name="red_in")
            red_out = dram.tile([128, 2 * NT], f32, name="red_out")

            groups = [list(range(NC8))]

            def body():
                # ---------------- loads
                ag1_in = dram.tile([2, MT, 128, FD], r32, name="ag1_in", tag="ag1_in")
                ag1_out = dram.tile([NC8, 2, MT, 128, FD], r32, name="ag1_out", tag="ag1_out", addr_space="Shared")
                ag2_in = dram.tile([MT, 128, FD], r32, name="ag2_in", tag="ag2_in")
                ag2_out = dram.tile([NT, 128, FD], r32, name="ag2_out", tag="ag2_out", addr_space="Shared")
                ag3_in = dram.tile([2, MT, 128, FD], r32, name="ag3_in", tag="ag3_in")
                ag3_out = dram.tile([NC8, 2, MT, 128, FD], r32, name="ag3_out", tag="ag3_out", addr_space="Shared")
                x0loc_sb = x0lp.tile([128, MT, FD], f32, tag="x0l", name="x0loc_sb")
                nc.sync.dma_start(out=acols_sb[:],
                                  in_=acols_d.ap().rearrange("t p m -> p t m"))
                nc.sync.dma_start(out=arowsT_sb[:],
                                  in_=arowsT_d.ap().rearrange("t p m -> p t m"))
                nc.sync.dma_start(out=hxtu_sb[0:U, :, :], in_=hxT_d.ap())
                nc.sync.dma_start(out=x0loc_sb[:],
                                  in_=x0loc_d.ap().rearrange("m p f -> p m f"))
                nc.sync.dma_start(out=wg_sb[:],
                                  in_=Wg_d.ap().rearrange("k p o -> p k o"))
                nc.sync.dma_start(out=wc_sb[:],
                                  in_=Wc_d.ap().rearrange("k p o -> p k o"))
                nc.sync.dma_start(out=bg_sb[:], in_=bg_d.ap())
                nc.sync.dma_start(out=bc_sb[:], in_=bc_d.ap())
                nc.sync.dma_start(out=ident[:], in_=ident_d.ap())
                nc.vector.memset(zero_sb[:], 0.0)

                # ---------------- degree sums -> dinv / d2inv, scale stationaries
                X = mybir.AxisListType.X
                for t in range(NT):
                    nc.vector.tensor_reduce(out=dred[:, t:t + 1],
                                            in_=acols_sb[:, t, :].bitcast(f32),
                                            axis=X, op=Alu.add)
                    nc.vector.tensor_reduce(out=dred[:, NT + t:NT + t + 1],
                                            in_=arowsT_sb[:, t, :].bitcast(f32),
                                            axis=X, op=Alu.add)
                nc.sync.dma_start(out=red_in[:], in_=dred[:])
                if not no_cc:
                    nc.gpsimd.collective_compute(
                        "AllReduce", Alu.add, replica_groups=groups,
                        ins=[red_in[:].opt()], outs=[red_out[:].opt()])
                nc.sync.dma_start(out=dtmp[:], in_=red_out[:])
                # dinv = (1/max(d,eps)) * (d > 0)
                nc.vector.tensor_scalar_max(dmask[:], dtmp[:], 1e-30)
                nc.vector.reciprocal(dinv_sb[:], dmask[:])
                nc.vector.tensor_scalar(out=dmask[:], in0=dtmp[:], scalar1=0.0,
                                        scalar2=None, op0=Alu.is_gt)
                nc.vector.tensor_tensor(out=dinv_sb[:], in0=dinv_sb[:],
                                        in1=dmask[:], op=Alu.mult)
                for t in range(NT):
                    nc.vector.tensor_scalar_mul(acols_sb[:, t, :],
                                                acols_sb[:, t, :],
                                                dinv_sb[:, t:t + 1])
                    nc.vector.tensor_scalar_mul(arowsT_sb[:, t, :],
                                                arowsT_sb[:, t, :],
                                                dinv_sb[:, NT + t:NT + t + 1])

                # ---------------- helpers
                def alloc_main_psums():
                    return [[psp.tile([128, CH], f32, name=f"pm{m}{q}",
                                      tag=f"pm{m}{q}", bufs=1)
                             for q in range(NCH)] for m in range(MT)]

                def aux_psum(name, free, dt=f32):
                    return psp.tile([128, free], dt, name=name, tag="aux", bufs=2)

                def diffusion_pass(lhsT_sb, rhs_ap_fn, combine):
                    """combine(m, c0, c1, psum_ap) writes [128, c1-c0] output."""
                    pm = alloc_main_psums()
                    ptail = aux_psum("ptail", MT * TAIL) if TAIL else None
                    for t in range(NT):
                        rt = rhsp.tile([128, FD], r32, name="rt", tag="rt")
                        nc.sync.dma_start(out=rt[:], in_=rhs_ap_fn(t))
                        st, sp = (t == 0), (t == NT - 1)
                        for m in range(MT):
                            lh = lhsT_sb[:, t, m * 128:(m + 1) * 128]
                            for q in range(NCH):
                                nc.tensor.matmul(out=pm[m][q][:], lhsT=lh,
                                                 rhs=rt[:, q * CH:(q + 1) * CH],
                                                 start=st, stop=sp)
                            if TAIL:
                                nc.tensor.matmul(
                                    out=ptail[:, m * TAIL:(m + 1) * TAIL], lhsT=lh,
                                    rhs=rt[:, NCH * CH:FD], start=st, stop=sp)
                    for m in range(MT):
                        for q in range(NCH):
                            combine(m, q * CH, (q + 1) * CH, pm[m][q][:])
                        if TAIL:
                            combine(m, NCH * CH, FD,
                                    ptail[:, m * TAIL:(m + 1) * TAIL])

                def gconv(g, x0full_ap_fn, x0loc_tile, w_sb, b_sb, act_fn, O,
                          agi, ago):
                    stats = [acols_sb, arowsT_sb]
                    # ---- step 1: x1_s = S_s @ x0   (local rows)
                    x1 = []
                    for s in range(2):
                        x1loc = matsp.tile([128, MT, FD], r32,
                                           name=f"x1loc{g}{s}", tag="mats")
                        def comb1(m, c0, c1, ps, x1loc=x1loc):
                            nc.vector.tensor_copy(x1loc[:, m, c0:c1], ps)
                        diffusion_pass(stats[s], x0full_ap_fn, comb1)
                        nc.sync.dma_start(out=agi[s].rearrange("m p f -> p m f"),
                                          in_=x1loc[:])
                        x1.append(x1loc)
                    # ---- allgather both supports' x1
                    if not no_cc:
                        nc.gpsimd.collective_compute(
                            "AllGather", Alu.bypass, replica_groups=groups,
                            ins=[agi[:].opt()], outs=[ago[:].opt()])
                    # ---- step 2: x2_s = 2 * S_s @ x1_s - x0  (local rows)
                    x2 = []
                    for s in range(2):
                        x2loc = matsp.tile([128, MT, FD], r32,
                                           name=f"x2loc{g}{s}", tag="mats")

                        def rhs2(t, s=s):
                            return ago[t // MT, s, t % MT, :, :]

                        def comb2(m, c0, c1, ps, x2loc=x2loc):
                            nc.vector.scalar_tensor_tensor(
                                out=x2loc[:, m, c0:c1], in0=ps, scalar=2.0,
                                in1=x0loc_tile[:, m, c0:c1],
                                op0=Alu.mult, op1=Alu.subtract)
                        diffusion_pass(stats[s], rhs2, comb2)
                        x2.append(x2loc)

                    # ---- per-b: transpose mats into xsT_b, project, activate
                    # xsT k-blocks (one per mat, c-padded to 128):
                    #   k=0: rows 0:U = state channels, rows U:C'=U+F = inputs
                    #   k>=1: rows 0:C = c in natural order
                    for b in range(B):
                        xsT = xstp.tile([128, KT3, RL], r32, name="xsT", tag="xsT")
                        for k in range(KT3):
                            nc.vector.tensor_copy(xsT[64:128, k, :],
                                                  zero_sb[64:128, :])
                        # k = 0 rows: x0T (state-first permuted layout, host-prepped)
                        if g == 0:
                            nc.sync.dma_start(out=xsT[0:C, 0, :],
                                              in_=x0T_d.ap()[:, b, :])
                        else:
                            nc.vector.tensor_copy(xsT[0:U, 0, :],
                                                  rhc_sb[0:U, b, :])
                            nc.sync.dma_start(out=xsT[U:C, 0, :],
                                              in_=x0T_d.ap()[U:C, b, :])
                        # mats 1..4: (x1 s0), (x2 s0), (x1 s1), (x2 s1)
                        matspec = [(1, "dram", 0), (2, "sbuf", 0),
                                   (3, "dram", 1), (4, "sbuf", 1)]
                        for k, kind, s in matspec:
                            for nb in range(MT):
                                if kind == "dram":
                                    tt = ttp.tile([128, C], r32, name="tt", tag="tt")
                                    nc.sync.dma_start(
                                        out=tt[:],
                                        in_=agi[s, nb, :, b * C:(b + 1) * C])
                                    src = tt[:]
                                else:
                                    src = x2[s][:, nb, b * C:(b + 1) * C]
                                pst = aux_psum("pst", 128, r32)
                                nc.tensor.transpose(pst[0:C, :], src, ident[:])
                                nc.vector.tensor_copy(
                                    xsT[0:C, k, nb * 128:(nb + 1) * 128],
                                    pst[0:C, :])
                        # projection: out_b^T [O, RL]
                        pso = aux_psum("pso", RL)
                        for kk in range(KT3):
                            nc.tensor.matmul(out=pso[0:O, :],
                                             lhsT=w_sb[:, kk, 0:O],
                                             rhs=xsT[:, kk, :],
                                             start=(kk == 0), stop=(kk == KT3 - 1))
                        if g == 0:
                            sg = sgp.tile([128, RL], f32, name="sg", tag="sg")
                            nc.scalar.activation(sg[:], pso[:], Act.Sigmoid,
                                                 bias=bg_sb[:])
                            # rh = r * hx ; stash u
                            nc.vector.tensor_tensor(out=rhc_sb[0:U, b, :],
                                                    in0=sg[0:U, :],
                                                    in1=hxtu_sb[0:U, b, :],
                                                    op=Alu.mult)
                            nc.vector.tensor_copy(hxtu_sb[U:128, b, :],
                                                  sg[U:128, :])
                        else:
                            cvw = rhc_sb[U:128, b, :]
                            nc.scalar.activation(cvw, pso[0:U, :], Act.Tanh,
                                                 bias=bc_sb[:])
                            # out = u*(hx - c) + c ; all operands at base
                            # partition 64 (DVE needs equal input bases)
                            t1 = ftp.tile([128, RL], f32, name="t1", tag="ft")
                            nc.sync.dma_start(out=t1[U:128, :],
                                              in_=hxT_d.ap()[:, b, :])
                            t2 = ftp.tile([128, RL], f32, name="t2", tag="ft")
                            nc.vector.tensor_tensor(out=t2[U:128, :],
                                                    in0=t1[U:128, :],
                                                    in1=cvw, op=Alu.subtract)
                            t3 = ftp.tile([128, RL], f32, name="t3", tag="ft")
                            nc.vector.tensor_tensor(out=t3[U:128, :],
                                                    in0=hxtu_sb[U:128, b, :],
                                                    in1=t2[U:128, :], op=Alu.mult)
                            t4 = ftp.tile([128, RL], f32, name="t4", tag="ft")
                            nc.vector.tensor_tensor(out=t4[U:128, :],
                                                    in0=t3[U:128, :],
                                                    in1=cvw, op=Alu.add)
                            nc.sync.dma_start(out=out_d.ap()[:, b, :],
                                              in_=t4[U:128, :])

                # ================ gconv 1 (gate)
                gconv(0, lambda t: x0full_d.ap()[t, :, :], x0loc_sb, wg_sb, bg_sb,
                      None, 2 * U, ag1_in, ag1_out)

                # ================ assemble x0' = concat(inputs, r*hx), gather
                # (x0ploc reuses x0loc's SBUF slot; input-feature columns come
                # straight from the x0loc DRAM input)
                x0ploc_sb = x0lp.tile([128, MT, FD], r32, tag="x0l")
                x0p4 = x0ploc_sb[:].rearrange("p m (b c) -> p m b c", c=C)
                for mi in range(MT):
                    nc.sync.dma_start(
                        out=x0p4[:, mi, :, 0:F],
                        in_=x0loc_d.ap().bitcast(r32).rearrange(
                            "m p (b c) -> p m b c", c=C)[:, mi, :, 0:F])
                for b in range(B):
                    for nb in range(MT):
                        pst = aux_psum("psr", 128, f32)
                        nc.tensor.transpose(
                            pst[:, 0:U],
                            rhc_sb[0:U, b, nb * 128:(nb + 1) * 128],
                            ident[0:U, 0:U].bitcast(f32))
                        nc.vector.tensor_copy(
                            x0ploc_sb[:, nb, b * C + F:(b + 1) * C], pst[:, 0:U])
                nc.sync.dma_start(out=ag2_in[:].rearrange("m p f -> p m f"),
                                  in_=x0ploc_sb[:])
                if not no_cc:
                    nc.gpsimd.collective_compute(
                        "AllGather", Alu.bypass, replica_groups=groups,
                        ins=[ag2_in[:].opt()], outs=[ag2_out[:].opt()])

                # ================ gconv 2 (candidate) + GRU output
                gconv(1, lambda t: ag2_out[t, :, :], x0ploc_sb, wc_sb, bc_sb,
                      None, U, ag3_in, ag3_out)


            for _rep in range(reps):
                body()
    nc.compile()
    return nc


# ---------------------------------------------------------------- host side

def host_prep(cfg: Cfg, inputs, hx, adj_mx, W_gate, b_gate, W_cand, b_cand):
    N, NP, B, C, U, F, FD = cfg.N, cfg.NP, cfg.B, cfg.C, cfg.U, cfg.F, cfg.FD
    NT, RL, MT, NC8 = cfg.NT, cfg.RL, cfg.MT, cfg.NCORES

    A = np.zeros((NP, NP), np.float32)
    A[:N, :N] = adj_mx
    AT = np.ascontiguousarray(A.T)

    xcat = np.concatenate([inputs.reshape(B, N, F).astype(np.float32),
                           hx.reshape(B, N, U).astype(np.float32)], axis=2)
    perm0 = np.concatenate([np.arange(F, C), np.arange(F)])  # state-first
    x0nat = np.zeros((NP, FD), np.float32)
    x0nat[:N] = xcat.transpose(1, 0, 2).reshape(N, FD)
    hxp = np.zeros((NP, B, U), np.float32)
    hxp[:N] = hx.reshape(B, N, U).transpose(1, 0, 2)

    # W~ packed into NMATS k-blocks of 128 rows (c-padded).
    # k=0 block is state-first permuted: row c' = c-F for c>=F, row U+c for c<F.
    KT3, NM = cfg.KT3, cfg.NMATS

    def packw(W, O):
        Wp = np.zeros((KT3, 128, O), np.float32)
        for k in range(NM):
            blk = W[np.arange(C) * NM + k]        # [C, O] rows c
            if k == 0:
                Wp[0, 0:U] = blk[F:C]
                Wp[0, U:C] = blk[0:F]
            else:
                Wp[k, 0:C] = blk
        return np.ascontiguousarray(Wp)

    Wg = packw(W_gate, 2 * U)
    Wc = packw(W_cand, U)
    bg = np.ascontiguousarray(b_gate.reshape(2 * U, 1).astype(np.float32))
    bc = np.ascontiguousarray(b_cand.reshape(U, 1).astype(np.float32))

    in_maps = []
    for c in range(NC8):
        sl = slice(c * RL, (c + 1) * RL)
        in_maps.append({
            "acols": np.ascontiguousarray(A[:, sl].reshape(NT, 128, RL)),
            "arowsT": np.ascontiguousarray(AT[:, sl].reshape(NT, 128, RL)),
            "x0full": np.ascontiguousarray(x0nat.reshape(NT, 128, FD)),
            "x0loc": np.ascontiguousarray(x0nat[sl].reshape(MT, 128, FD)),
            "x0T": np.ascontiguousarray(
                x0nat[sl].reshape(RL, B, C)[:, :, perm0].transpose(2, 1, 0)),
            "hxT": np.ascontiguousarray(hxp[sl].transpose(2, 1, 0)),
            "Wg": Wg, "Wc": Wc, "bg": bg, "bc": bc,
            "ident": np.eye(128, dtype=np.float32),
        })
    return in_maps


def host_post(cfg: Cfg, results):
    N, B, U, RL = cfg.N, cfg.B, cfg.U, cfg.RL
    full = np.concatenate([results[c]["out"].transpose(2, 1, 0)[None]
                           for c in range(cfg.NCORES)], axis=0)  # [8, RL, B, U]
    full = full.reshape(cfg.NP, B, U)[:N]          # [N, B, U]
    return np.ascontiguousarray(full.transpose(1, 0, 2).reshape(B, N * U))


# ---------------------------------------------------------------- runner

class SpmdRunner:
    def __init__(self, nc, n_cores: int):
        import jax
        import jax.numpy as jnp
        from jax.sharding import Mesh, PartitionSpec, NamedSharding
        from jax.experimental.shard_map import shard_map
        import concourse.mybir as mybir
        from concourse.bass2jax import (_bass_exec_p, install_neuronx_cc_hook,
                                        partition_id_tensor)
        self.jax = jax
        install_neuronx_cc_hook()
        self.nc = nc
        self.n_cores = n_cores
        partition_name = (nc.partition_id_tensor.name
                          if nc.partition_id_tensor else None)
        dbg_name = nc.dbg_addr.name if nc.dbg_addr is not None else None
        in_names, out_names, out_avals = [], [], []
        for alloc in nc.m.functions[0].allocations:
            if not isinstance(alloc, mybir.MemoryLocationSet):
                continue
            name = alloc.memorylocations[0].name
            if alloc.kind == "ExternalInput":
                if name not in (partition_name, dbg_name):
                    in_names.append(name)
            elif alloc.kind == "ExternalOutput":
                out_avals.append(jax.core.ShapedArray(
                    tuple(alloc.tensor_shape), mybir.dt.np(alloc.dtype)))
                out_names.append(name)
        self.in_names, self.out_names, self.out_avals = (in_names, out_names,
                                                         out_avals)
        n_params, n_outs = len(in_names), len(out_names)
        all_in_names = list(in_names) + list(out_names)
        if dbg_name is not None:
            all_in_names.append(dbg_name)
        if partition_name is not None:
            all_in_names.append(partition_name)
        self._has_dbg = dbg_name is not None

        def _body(*args):
            operands = list(args)
            if partition_name is not None:
                operands.append(partition_id_tensor())
            return tuple(_bass_exec_p.bind(
                *operands, out_avals=tuple(out_avals),
                in_names=tuple(all_in_names), out_names=tuple(out_names),
                lowering_input_output_aliases=(),
                sim_require_finite=True, sim_require_nnan=True, nc=nc))

        try:
            devices = jax.devices("axon")[:n_cores]
        except RuntimeError:
            devices = jax.devices()[:n_cores]
        assert len(devices) == n_cores, f"need {n_cores} devices"
        self.mesh = Mesh(np.asarray(devices), ("core",))
        self.sharding = NamedSharding(self.mesh, PartitionSpec("core"))
        n_extra = 1 if self._has_dbg else 0
        in_specs = (PartitionSpec("core"),) * (n_params + n_outs + n_extra)
        out_specs = (PartitionSpec("core"),) * n_outs
        donate = tuple(range(n_params, n_params + n_outs))
        self.fn = jax.jit(
            shard_map(_body, mesh=self.mesh, in_specs=in_specs,
                      out_specs=out_specs, check_rep=False),
            donate_argnums=donate, keep_unused=True)

        def _mkzeros():
            zs = [jnp.zeros((n_cores * av.shape[0], *av.shape[1:]), av.dtype)
                  for av in out_avals]
            if self._has_dbg:
                zs.append(jnp.zeros((n_cores, 2), jnp.uint32))
            return tuple(zs)
        self.mkzeros = jax.jit(
            _mkzeros, out_shardings=(self.sharding,) * (n_outs + n_extra))
        self._dev_in = None

    def set_inputs(self, in_maps):
        concat = [np.ascontiguousarray(np.concatenate(
            [np.asarray(in_maps[c][name]) for c in range(self.n_cores)], axis=0))
            for name in self.in_names]
        self._dev_in = [self.jax.device_put(a, self.sharding) for a in concat]
        self.jax.block_until_ready(self._dev_in)

    def run(self):
        zeros = self.mkzeros()
        self.jax.block_until_ready(zeros)
        t0 = time.perf_counter()
        outs = self.fn(*self._dev_in, *zeros)
        self.jax.block_until_ready(outs)
        self.last_wall = time.perf_counter() - t0
        return outs

    def results(self, outs):
        return [{name: np.asarray(outs[i]).reshape(
            self.n_cores, *self.out_avals[i].shape)[c]
            for i, name in enumerate(self.out_names)}
            for c in range(self.n_cores)]


# ---------------------------------------------------------------- entry point

_CACHE = {}


def _get_runner():
    if "runner" not in _CACHE:
        nc = build_nc(CFG)
        _CACHE["runner"] = SpmdRunner(nc, CFG.NCORES)
    return _CACHE["runner"]


def kernel(inputs, hx, adj_mx, W_gate, b_gate, W_cand, b_cand, num_nodes=None):
    inputs, hx, adj_mx, W_gate, b_gate, W_cand, b_cand = [
        np.asarray(a, np.float32)
        for a in (inputs, hx, adj_mx, W_gate, b_gate, W_cand, b_cand)]
    r = _get_runner()
    in_maps = host_prep(CFG, inputs, hx, adj_mx, W_gate, b_gate, W_cand,
                        b_cand)
    r.set_inputs(in_maps)
    outs = r.run()
    return host_post(CFG, r.results(outs))



# revision 15
# speedup vs baseline: 3.4857x; 3.4857x over previous
"""DCGRU cell on 8 Trainium2 NeuronCores (Bass/Tile SPMD kernel).

Strategy (node sharding, bf16 compute):
  - Nodes padded 3000->3072, sharded 8x384 rows per core.
  - The two random-walk supports are never materialized; the host
    pre-scales the adjacency slices by the (global) inverse degrees:
      acols[n, j]  = A[n, cRL+j] * dinv[n]    (lhsT for S1-type products)
      arowsT[n, j] = A[cRL+j, n] * d2inv[n]   (lhsT for S2-type products)
    both stored bf16 and SBUF-resident.
  - Diffusion: out[m,cb] = sum_n lhsT[n,m] * rhs[n,cb] with rhs the full
    x tensor [3072, 1056] (bf16) streamed k-tile by k-tile from DRAM;
    Chebyshev step 2 needs full x1 -> AllGather (bf16) between steps.
  - Activations layout: natural [node, (b,c)] with col = b*66+c.  The
    projection contracts over (c,k-mat): per-(b, mat) 128x66 blocks are
    transposed on the PE into a resident xsT [128, 5, B, 384] (bf16),
    then host-packed W~ (5 k-tiles of 128 rows, zero pad rows) projects
    in 5 matmuls per b.  PSUM->SBUF copies are spread over DVE /
    Pool(gpsimd) / Activation engines.
  - x1 flips are emitted between the AllGather and diffusion step 2 so
    the PE does them inside the collective window.
  - GRU output math runs at partition base 64 (u | hx-dup | c all base
    64) to satisfy engine base-alignment; host duplicates hx into rows
    64:128 of hx2.
"""
import sys
import time

for _p in ("/opt/trn_rl_repo",):
    if _p not in sys.path:
        sys.path.insert(0, _p)

import numpy as np


# ---------------------------------------------------------------- config

class Cfg:
    def __init__(self, N=3000, NP=3072, B=16, F=2, U=64, NCORES=8):
        self.N, self.NP, self.B, self.F, self.U, self.NCORES = N, NP, B, F, U, NCORES
        self.C = F + U                    # 66
        self.FD = self.B * self.C         # 1056
        self.NT = NP // 128               # k tiles over nodes
        self.RL = NP // NCORES            # local rows per core
        self.MT = self.RL // 128          # local m tiles
        assert NP % 128 == 0 and self.RL % 128 == 0
        self.NMATS = 5                    # x0, x1s1, x2s1, x1s2, x2s2
        self.KT = self.NMATS              # one 128-row k-block per mat
        # free chunking of FD for diffusion matmuls: 512,512,tail
        self.CH = 512
        self.NCH = (self.FD // self.CH)           # 2 full chunks
        self.TAIL = self.FD - self.NCH * self.CH  # 32


CFG = Cfg()


# ---------------------------------------------------------------- device build

def build_nc(cfg: Cfg, reps: int = 1, no_cc: bool = False):
    import concourse.bass as bass
    import concourse.mybir as mybir
    import concourse.tile as tile
    from concourse import bacc

    bf16 = mybir.dt.bfloat16
    f32 = mybir.dt.float32
    Alu = mybir.AluOpType
    Act = mybir.ActivationFunctionType

    NP, NT, RL, MT, B, C, U, F, FD = (cfg.NP, cfg.NT, cfg.RL, cfg.MT, cfg.B,
                                      cfg.C, cfg.U, cfg.F, cfg.FD)
    CH, NCH, TAIL = cfg.CH, cfg.NCH, cfg.TAIL
    KT = cfg.KT
    NC8 = cfg.NCORES

    nc = bacc.Bacc("TRN2", target_bir_lowering=False, debug=False,
                   num_devices=NC8)

    # external inputs (per core)
    acols_d = nc.dram_tensor("acols", [NT, 128, RL], bf16, kind="ExternalInput")
    arowsT_d = nc.dram_tensor("arowsT", [NT, 128, RL], bf16, kind="ExternalInput")
    x0full_d = nc.dram_tensor("x0full", [NT, 128, FD], bf16, kind="ExternalInput")
    x0loc_d = nc.dram_tensor("x0loc", [MT, 128, FD], bf16, kind="ExternalInput")
    x0T_d = nc.dram_tensor("x0T", [C, B, RL], bf16, kind="ExternalInput")
    hx2_d = nc.dram_tensor("hx2", [128, B, RL], f32, kind="ExternalInput")
    Wg_d = nc.dram_tensor("Wg", [KT, 128, 2 * U], bf16, kind="ExternalInput")
    Wc_d = nc.dram_tensor("Wc", [KT, 128, U], bf16, kind="ExternalInput")
    bg_d = nc.dram_tensor("bg", [2 * U, 1], f32, kind="ExternalInput")
    bc_d = nc.dram_tensor("bc", [U, 1], f32, kind="ExternalInput")
    ident_d = nc.dram_tensor("ident", [128, 128], bf16, kind="ExternalInput")
    out_d = nc.dram_tensor("out", [U, B, RL], f32, kind="ExternalOutput")

    with tile.TileContext(nc) as tc:
        import contextlib
        ctx = contextlib.ExitStack()
        with ctx:
            const = ctx.enter_context(tc.tile_pool(name="const", bufs=1))
            matsp = ctx.enter_context(tc.tile_pool(name="mats", bufs=4))
            rhsp = ctx.enter_context(tc.tile_pool(name="rhsp", bufs=3))
            sgp = ctx.enter_context(tc.tile_pool(name="sgp", bufs=2))
            ftp = ctx.enter_context(tc.tile_pool(name="ftp", bufs=2))
            x0lp = ctx.enter_context(tc.tile_pool(name="x0lp", bufs=1))
            xstp = ctx.enter_context(tc.tile_pool(name="xstp", bufs=1))
            psp = ctx.enter_context(tc.tile_pool(name="psp", bufs=1, space="PSUM"))
            dram = ctx.enter_context(tc.tile_pool(name="dram", bufs=reps, space="DRAM"))

            # ---------------- resident SBUF tensors (buffers persist; data
            # reloaded each rep inside body())
            acols_sb = const.tile([128, NT, RL], bf16)
            arowsT_sb = const.tile([128, NT, RL], bf16)
            hx2_sb = const.tile([128, B, RL], f32)      # rows 0:U and U:128 hxT
            u_sb = const.tile([128, B, RL], bf16)       # rows U:128 = u
            rhc_sb = const.tile([128, B, RL], bf16)     # rows 0:U = r*hx
            wg_sb = const.tile([128, KT, 2 * U], bf16)
            wc_sb = const.tile([128, KT, U], bf16)
            bg_sb = const.tile([2 * U, 1], f32)
            bc_sb = const.tile([U, 1], f32)
            ident = const.tile([128, 128], bf16)

            groups = [list(range(NC8))]
            copy_engines = [nc.vector, nc.gpsimd, nc.scalar]

            def ecopy(i, out, in_):
                eng = copy_engines[i % len(copy_engines)]
                if eng is nc.scalar:
                    eng.copy(out, in_)
                else:
                    eng.tensor_copy(out, in_)

            def body():
                ag1_in = dram.tile([2, MT, 128, FD], bf16, name="ag1_in", tag="ag1_in")
                ag1_out = dram.tile([NC8, 2, MT, 128, FD], bf16, name="ag1_out",
                                    tag="ag1_out", addr_space="Shared")
                ag2_in = dram.tile([MT, 128, FD], bf16, name="ag2_in", tag="ag2_in")
                ag2_out = dram.tile([NT, 128, FD], bf16, name="ag2_out",
                                    tag="ag2_out", addr_space="Shared")
                ag3_in = dram.tile([2, MT, 128, FD], bf16, name="ag3_in", tag="ag3_in")
                ag3_out = dram.tile([NC8, 2, MT, 128, FD], bf16, name="ag3_out",
                                    tag="ag3_out", addr_space="Shared")
                x0p_sb = x0lp.tile([128, MT, FD], bf16, tag="x0l", name="x0p_sb")
                xsT = xstp.tile([128, KT, B, RL], bf16, tag="xsT", name="xsT")

                # ---------------- loads: only ident up front; the stationary
                # A tiles stream inside gconv1's step-1 t-loops just ahead of
                # each rhs tile (all DMAs share one queue), and everything
                # else is emitted after step 1 so the PE starts immediately.
                nc.sync.dma_start(out=ident[:], in_=ident_d.ap())
                stat_d = [acols_d, arowsT_d]

                def late_loads():
                    nc.sync.dma_start(out=x0p_sb[:],
                                      in_=x0loc_d.ap().rearrange("m p f -> p m f"))
                    nc.sync.dma_start(out=xsT[0:C, 0, :, :], in_=x0T_d.ap())
                    nc.sync.dma_start(out=hx2_sb[:], in_=hx2_d.ap())
                    nc.sync.dma_start(out=wg_sb[:],
                                      in_=Wg_d.ap().rearrange("k p o -> p k o"))
                    nc.sync.dma_start(out=wc_sb[:],
                                      in_=Wc_d.ap().rearrange("k p o -> p k o"))
                    nc.sync.dma_start(out=bg_sb[:], in_=bg_d.ap())
                    nc.sync.dma_start(out=bc_sb[:], in_=bc_d.ap())
                # zero the xsT pad partitions once (W pad rows are zero, but
                # 0 * garbage-NaN would still poison the psum)
                nc.vector.memset(xsT[U:128, 0:3, :, :], 0.0)
                nc.gpsimd.memset(xsT[U:128, 3:KT, :, :], 0.0)

                stats = [acols_sb, arowsT_sb]

                def diffusion(rhs_fn, combine, load_stat=False):
                    """One support-pass: psum[m] over all k-tiles, then
                    combine(m, c0, c1, psum_ap)."""
                    pm = [[psp.tile([128, CH], f32, name=f"pm{m}{q}",
                                    tag=f"pm{m}{q}", bufs=1)
                           for q in range(NCH)] for m in range(MT)]
                    ptail = psp.tile([128, MT * TAIL], f32, name="ptail",
                                     tag="aux", bufs=2)
                    s = combine.s
                    lhsT_sb = stats[s]
                    for t in range(NT):
                        if load_stat:
                            nc.sync.dma_start(out=lhsT_sb[:, t, :],
                                              in_=stat_d[s].ap()[t, :, :])
                        rt = rhsp.tile([128, FD], bf16, name="rt", tag="rt")
                        nc.sync.dma_start(out=rt[:], in_=rhs_fn(t))
                        st, sp = (t == 0), (t == NT - 1)
                        for m in range(MT):
                            lh = lhsT_sb[:, t, m * 128:(m + 1) * 128]
                            for q in range(NCH):
                                nc.tensor.matmul(out=pm[m][q][:], lhsT=lh,
                                                 rhs=rt[:, q * CH:(q + 1) * CH],
                                                 start=st, stop=sp)
                            nc.tensor.matmul(
                                out=ptail[:, m * TAIL:(m + 1) * TAIL], lhsT=lh,
                                rhs=rt[:, NCH * CH:FD], start=st, stop=sp)
                    for m in range(MT):
                        for q in range(NCH):
                            combine(m, q * CH, (q + 1) * CH, pm[m][q][:])
                        combine(m, NCH * CH, FD,
                                ptail[:, m * TAIL:(m + 1) * TAIL])

                def flip2(srcs, b, ks, ci):
                    """Transpose both mats' [128, C] blocks of b into one
                    single-bank psum tile, drain with one strided copy per
                    mat (GPSIMD cannot touch PSUM: DVE/Act only)."""
                    pst = psp.tile([128, 2, MT, 128], bf16, name="pst",
                                   tag="aux", bufs=2)
                    for si in range(2):
                        for nb in range(MT):
                            nc.tensor.transpose(
                                pst[0:C, si, nb, :],
                                srcs[si][:, nb, b * C:(b + 1) * C], ident[:])
                    for si in range(2):
                        nc.vector.tensor_copy(xsT[0:C, ks[si], b, :],
                                              pst[0:C, si, :, :])

                def gconv(g, rhs1_fn, w_sb, b_sb, O, agi, ago):
                    # ---- step 1: x1_s = S_s @ x0
                    x1 = []
                    for s in range(2):
                        x1loc = matsp.tile([128, MT, FD], bf16,
                                           name=f"x1loc{g}{s}", tag="mats")
                        ev = nc.vector

                        def comb1(m, c0, c1, ps, x1loc=x1loc, ev=ev):
                            if ev is nc.scalar:
                                ev.copy(x1loc[:, m, c0:c1], ps)
                            else:
                                ev.tensor_copy(x1loc[:, m, c0:c1], ps)
                        comb1.s = s
                        diffusion(rhs1_fn, comb1, load_stat=(g == 0))
                        nc.sync.dma_start(out=agi[s].rearrange("m p f -> p m f"),
                                          in_=x1loc[:])
                        x1.append(x1loc)
                    if g == 0:
                        late_loads()
                    # ---- allgather both supports' x1
                    if not no_cc:
                        nc.gpsimd.collective_compute(
                            "AllGather", Alu.bypass, replica_groups=groups,
                            ins=[agi[:].opt()], outs=[ago[:].opt()])
                    # ---- x1 flips run on PE inside the collective window
                    for b in range(B):
                        flip2(x1, b, (1, 3), b)
                    # ---- step 2: x2_s = 2 * S_s @ x1_s - x0
                    x2 = []
                    for s in range(2):
                        x2loc = matsp.tile([128, MT, FD], bf16,
                                           name=f"x2loc{g}{s}", tag="mats")
                        def rhs2(t, s=s):
                            return ago[t // MT, s, t % MT, :, :]

                        def comb2(m, c0, c1, ps, x2loc=x2loc):
                            nc.vector.scalar_tensor_tensor(
                                out=x2loc[:, m, c0:c1], in0=ps, scalar=2.0,
                                in1=x0p_sb[:, m, c0:c1],
                                op0=Alu.mult, op1=Alu.subtract)
                        comb2.s = s
                        diffusion(rhs2, comb2)
                        x2.append(x2loc)

                    # ---- per-b: flip x2 mats, project, activate, combine
                    for b in range(B):
                        flip2(x2, b, (2, 4), b)
                        pso = psp.tile([128, RL], f32, name="pso", tag="aux",
                                       bufs=2)
                        for kk in range(KT):
                            nc.tensor.matmul(out=pso[0:O, :],
                                             lhsT=w_sb[:, kk, 0:O],
                                             rhs=xsT[:, kk, b, :],
                                             start=(kk == 0), stop=(kk == KT - 1))
                        if g == 0:
                            sg = sgp.tile([128, RL], f32, name="sg", tag="sg")
                            nc.scalar.activation(sg[:], pso[:], Act.Sigmoid,
                                                 bias=bg_sb[:])
                            # rh = r * hx ; stash u; k0 state rows for gconv2
                            # (Pool engine handles the SBUF-only ops)
                            nc.gpsimd.tensor_tensor(out=rhc_sb[0:U, b, :],
                                                    in0=sg[0:U, :],
                                                    in1=hx2_sb[0:U, b, :],
                                                    op=Alu.mult)
                            nc.gpsimd.tensor_copy(u_sb[U:128, b, :],
                                                  sg[U:128, :])
                            nc.gpsimd.tensor_copy(xsT[0:U, 0, b, :],
                                                  rhc_sb[0:U, b, :])
                            # x0' state cols in natural layout (in-place);
                            # batch the 3 node-blocks into one psum tile
                            pstb = psp.tile([128, MT, U], bf16, name="pstb",
                                            tag="aux", bufs=2)
                            for nb in range(MT):
                                nc.tensor.transpose(
                                    pstb[:, nb, :],
                                    rhc_sb[0:U, b, nb * 128:(nb + 1) * 128],
                                    ident[0:U, 0:U])
                            dst = x0p_sb[:].rearrange(
                                "p m (b c) -> p m b c", c=C)[:, :, b, F:C]
                            nc.vector.tensor_copy(dst, pstb[:])
                        else:
                            cv = sgp.tile([128, RL], f32, name="cv", tag="sg")
                            nc.scalar.activation(cv[U:128, :], pso[0:U, :],
                                                 Act.Tanh, bias=bc_sb[:])
                            # out = u*(hx - c) + c   (all at base partition 64)
                            ft1 = ftp.tile([128, RL], f32, name="ft1", tag="ft")
                            nc.vector.tensor_tensor(out=ft1[U:128, :],
                                                    in0=hx2_sb[U:128, b, :],
                                                    in1=cv[U:128, :],
                                                    op=Alu.subtract)
                            ft2 = ftp.tile([128, RL], f32, name="ft2", tag="ft")
                            nc.gpsimd.tensor_tensor(out=ft2[U:128, :],
                                                    in0=u_sb[U:128, b, :],
                                                    in1=ft1[U:128, :],
                                                    op=Alu.mult)
                            ft3 = ftp.tile([128, RL], f32, name="ft3", tag="ft")
                            nc.vector.tensor_tensor(out=ft3[U:128, :],
                                                    in0=ft2[U:128, :],
                                                    in1=cv[U:128, :],
                                                    op=Alu.add)
                            nc.sync.dma_start(out=out_d.ap()[:, b, :],
                                              in_=ft3[U:128, :])

                # ================ gconv 1 (gate)
                gconv(0, lambda t: x0full_d.ap()[t, :, :], wg_sb, bg_sb,
                      2 * U, ag1_in, ag1_out)

                # ================ gather x0' (input cols kept, state cols
                # overwritten in place during gconv1's per-b loop)
                nc.sync.dma_start(out=ag2_in[:].rearrange("m p f -> p m f"),
                                  in_=x0p_sb[:])
                if not no_cc:
                    nc.gpsimd.collective_compute(
                        "AllGather", Alu.bypass, replica_groups=groups,
                        ins=[ag2_in[:].opt()], outs=[ag2_out[:].opt()])

                # ================ gconv 2 (candidate) + GRU output
                gconv(1, lambda t: ag2_out[t, :, :], wc_sb, bc_sb,
                      U, ag3_in, ag3_out)

            for _rep in range(reps):
                body()
    nc.compile()
    return nc


# ---------------------------------------------------------------- host side

def host_prep(cfg: Cfg, inputs, hx, adj_mx, W_gate, b_gate, W_cand, b_cand):
    import ml_dtypes
    bf16 = ml_dtypes.bfloat16
    N, NP, B, C, U, F, FD = cfg.N, cfg.NP, cfg.B, cfg.C, cfg.U, cfg.F, cfg.FD
    NT, RL, MT, NC8 = cfg.NT, cfg.RL, cfg.MT, cfg.NCORES

    A = np.zeros((NP, NP), np.float32)
    A[:N, :N] = adj_mx
    d = A.sum(axis=1)
    dinv = np.where(d > 0, 1.0 / np.maximum(d, 1e-30), 0.0).astype(np.float32)
    d2 = A.sum(axis=0)
    d2inv = np.where(d2 > 0, 1.0 / np.maximum(d2, 1e-30), 0.0).astype(np.float32)
    Asc = (A * dinv[:, None]).astype(bf16)                  # lhsT for S1
    ATsc = (A.T * d2inv[:, None]).astype(bf16)              # lhsT for S2

    xcat = np.concatenate([inputs.reshape(B, N, F).astype(np.float32),
                           hx.reshape(B, N, U).astype(np.float32)], axis=2)
    perm0 = np.concatenate([np.arange(F, C), np.arange(F)])  # state-first
    x0nat = np.zeros((NP, FD), np.float32)
    x0nat[:N] = xcat.transpose(1, 0, 2).reshape(N, FD)
    x0bf = x0nat.astype(bf16)
    hxp = np.zeros((NP, B, U), np.float32)
    hxp[:N] = hx.reshape(B, N, U).transpose(1, 0, 2)

    # W~ packed into NMATS k-blocks of 128 rows (c-padded, zero pads).
    # k=0 block is state-first permuted: row r<U -> c=F+r, row U+i -> c=i.
    KT, NM = cfg.KT, cfg.NMATS

    def packw(W, O):
        Wp = np.zeros((KT, 128, O), np.float32)
        for k in range(NM):
            blk = W[np.arange(C) * NM + k]        # [C, O] rows c
            if k == 0:
                Wp[0, 0:U] = blk[F:C]
                Wp[0, U:C] = blk[0:F]
            else:
                Wp[k, 0:C] = blk
        return np.ascontiguousarray(Wp.astype(bf16))

    Wg = packw(W_gate, 2 * U)
    Wc = packw(W_cand, U)
    bg = np.ascontiguousarray(b_gate.reshape(2 * U, 1).astype(np.float32))
    bc = np.ascontiguousarray(b_cand.reshape(U, 1).astype(np.float32))

    in_maps = []
    for c in range(NC8):
        sl = slice(c * RL, (c + 1) * RL)
        hxT = hxp[sl].transpose(2, 1, 0)                     # [U, B, RL]
        hx2 = np.concatenate([hxT, hxT], axis=0)             # [128, B, RL]
        in_maps.append({
            "acols": np.ascontiguousarray(Asc[:, sl].reshape(NT, 128, RL)),
            "arowsT": np.ascontiguousarray(ATsc[:, sl].reshape(NT, 128, RL)),
            "x0full": np.ascontiguousarray(x0bf.reshape(NT, 128, FD)),
            "x0loc": np.ascontiguousarray(x0bf[sl].reshape(MT, 128, FD)),
            "x0T": np.ascontiguousarray(
                x0nat[sl].reshape(RL, B, C)[:, :, perm0]
                .transpose(2, 1, 0).astype(bf16)),
            "hx2": np.ascontiguousarray(hx2),
            "Wg": Wg, "Wc": Wc, "bg": bg, "bc": bc,
            "ident": np.eye(128, dtype=bf16),
        })
    return in_maps


def host_post(cfg: Cfg, results):
    N, B, U, RL = cfg.N, cfg.B, cfg.U, cfg.RL
    full = np.concatenate([results[c]["out"].transpose(2, 1, 0)[None]
                           for c in range(cfg.NCORES)], axis=0)  # [8, RL, B, U]
    full = full.reshape(cfg.NP, B, U)[:N]          # [N, B, U]
    return np.ascontiguousarray(full.transpose(1, 0, 2).reshape(B, N * U))


# ---------------------------------------------------------------- runner

class SpmdRunner:
    def __init__(self, nc, n_cores: int):
        import jax
        import jax.numpy as jnp
        from jax.sharding import Mesh, PartitionSpec, NamedSharding
        from jax.experimental.shard_map import shard_map
        import concourse.mybir as mybir
        from concourse.bass2jax import (_bass_exec_p, install_neuronx_cc_hook,
                                        partition_id_tensor)
        self.jax = jax
        install_neuronx_cc_hook()
        self.nc = nc
        self.n_cores = n_cores
        partition_name = (nc.partition_id_tensor.name
                          if nc.partition_id_tensor else None)
        dbg_name = nc.dbg_addr.name if nc.dbg_addr is not None else None
        in_names, out_names, out_avals = [], [], []
        for alloc in nc.m.functions[0].allocations:
            if not isinstance(alloc, mybir.MemoryLocationSet):
                continue
            name = alloc.memorylocations[0].name
            if alloc.kind == "ExternalInput":
                if name not in (partition_name, dbg_name):
                    in_names.append(name)
            elif alloc.kind == "ExternalOutput":
                out_avals.append(jax.core.ShapedArray(
                    tuple(alloc.tensor_shape), mybir.dt.np(alloc.dtype)))
                out_names.append(name)
        self.in_names, self.out_names, self.out_avals = (in_names, out_names,
                                                         out_avals)
        n_params, n_outs = len(in_names), len(out_names)
        all_in_names = list(in_names) + list(out_names)
        if dbg_name is not None:
            all_in_names.append(dbg_name)
        if partition_name is not None:
            all_in_names.append(partition_name)
        self._has_dbg = dbg_name is not None

        def _body(*args):
            operands = list(args)
            if partition_name is not None:
                operands.append(partition_id_tensor())
            return tuple(_bass_exec_p.bind(
                *operands, out_avals=tuple(out_avals),
                in_names=tuple(all_in_names), out_names=tuple(out_names),
                lowering_input_output_aliases=(),
                sim_require_finite=True, sim_require_nnan=True, nc=nc))

        try:
            devices = jax.devices("axon")[:n_cores]
        except RuntimeError:
            devices = jax.devices()[:n_cores]
        assert len(devices) == n_cores, f"need {n_cores} devices"
        self.mesh = Mesh(np.asarray(devices), ("core",))
        self.sharding = NamedSharding(self.mesh, PartitionSpec("core"))
        n_extra = 1 if self._has_dbg else 0
        in_specs = (PartitionSpec("core"),) * (n_params + n_outs + n_extra)
        out_specs = (PartitionSpec("core"),) * n_outs
        donate = tuple(range(n_params, n_params + n_outs))
        self.fn = jax.jit(
            shard_map(_body, mesh=self.mesh, in_specs=in_specs,
                      out_specs=out_specs, check_rep=False),
            donate_argnums=donate, keep_unused=True)

        def _mkzeros():
            zs = [jnp.zeros((n_cores * av.shape[0], *av.shape[1:]), av.dtype)
                  for av in out_avals]
            if self._has_dbg:
                zs.append(jnp.zeros((n_cores, 2), jnp.uint32))
            return tuple(zs)
        self.mkzeros = jax.jit(
            _mkzeros, out_shardings=(self.sharding,) * (n_outs + n_extra))
        self._dev_in = None

    def set_inputs(self, in_maps):
        concat = [np.ascontiguousarray(np.concatenate(
            [np.asarray(in_maps[c][name]) for c in range(self.n_cores)], axis=0))
            for name in self.in_names]
        self._dev_in = [self.jax.device_put(a, self.sharding) for a in concat]
        self.jax.block_until_ready(self._dev_in)

    def run(self):
        zeros = self.mkzeros()
        self.jax.block_until_ready(zeros)
        t0 = time.perf_counter()
        outs = self.fn(*self._dev_in, *zeros)
        self.jax.block_until_ready(outs)
        self.last_wall = time.perf_counter() - t0
        return outs

    def results(self, outs):
        return [{name: np.asarray(outs[i]).reshape(
            self.n_cores, *self.out_avals[i].shape)[c]
            for i, name in enumerate(self.out_names)}
            for c in range(self.n_cores)]


# ---------------------------------------------------------------- entry point

_CACHE = {}


def _get_runner():
    if "runner" not in _CACHE:
        nc = build_nc(CFG)
        _CACHE["runner"] = SpmdRunner(nc, CFG.NCORES)
    return _CACHE["runner"]


def kernel(inputs, hx, adj_mx, W_gate, b_gate, W_cand, b_cand, num_nodes=None):
    inputs, hx, adj_mx, W_gate, b_gate, W_cand, b_cand = [
        np.asarray(a, np.float32)
        for a in (inputs, hx, adj_mx, W_gate, b_gate, W_cand, b_cand)]
    r = _get_runner()
    in_maps = host_prep(CFG, inputs, hx, adj_mx, W_gate, b_gate, W_cand,
                        b_cand)
    r.set_inputs(in_maps)
    outs = r.run()
    return host_post(CFG, r.results(outs))
